# revision 3
# baseline (speedup 1.0000x reference)
"""CenterPool Trainium2 kernel.

Reference semantics (per bbox):
    img_xc = x + floor(w/2); img_yc = y + floor(h/2)
    cell_x = clip(floor(img_xc/8), 0, 63); cell_y likewise (cell=8px, fm 64x64)
    fv     = input[img_idx, :, cell_y, cell_x]                  # [*, 256]
    label  = [img_xc/8 - cell_x, img_yc/8 - cell_y, w/512, h/512]
    out    = fv + label @ W.T + b

Sharding: data-parallel over batch B=8 across 8 cores (one program, SPMD).
Core b receives input[4b:4b+4] staged CHANNEL-LAST ([K, FM, FM, C], NHWC) so
each box's 256-channel feature vector is one contiguous 1 KiB run in HBM.
Everything else rides in ONE packed [2, 1026] stage tensor per core:
transposed bbox components, W^T column blocks (rows 2,3 pre-scaled by 1/512
so the w/h label normalization folds into the matmul), the bias row, a
staged ones row, the per-box image-base row (k*2^20), and the offset-matmul
coefficient columns. One DMA per iteration loads it all; compute reads are
free-dim slices starting at partition 0 (compute APs must start 32-aligned,
so tiles are never partition-sliced except at 0).

The gather is a single gpsimd indirect DMA: a [64, 1] i32 offset table in
SBUF supplies one flat element offset per destination partition, and each
partition pulls its contiguous 1024 B feature vector (HW semantics: one
offset per partition, contiguous payload). Offsets are computed as
256*cx + 16384*cy + 2^20*k by a K=2+K=1 accumulating PE matmul over the
clipped cell rows and the staged kbase row -- all operands are small ints or
powers of two, so the fp32 matmul is exact -- then converted to i32.

The cell/label math runs batched in [2, 64] component-major tiles on DVE;
floor is the exact-IEEE 2^23 round-magic plus an is_gt correction. The label
linear is three accumulating K<=2 matmuls into a [64, 256] PSUM; DVE adds
the gathered features and one DMA stores the result.
"""

import sys

import numpy as np

sys.path.insert(0, "/opt/trn_rl_repo")

from concourse import bacc, bass, mybir, tile  # noqa: E402
from concourse import bass_utils  # noqa: E402

B, K, N, C = 8, 4, 16, 256
FM = 64
HW = FM * FM  # 4096 spatial positions per image
NBOX = K * N  # 64 boxes per core
NCORES = 8
NELEM = K * HW * C  # elements per core shard (channel-last layout)
MAGIC = 8388608.0  # 2^23: (v + MAGIC) - MAGIC rounds f32 to nearest int

# stage-tensor column layout ([2, ST_COLS] f32)
ST_XY = 0                 # cols    0:64   row0 = x, row1 = y
ST_WH = ST_XY + NBOX      # cols   64:128  row0 = w, row1 = h
ST_W01 = ST_WH + NBOX     # cols  128:384  rows 0:2 = W^T rows 0,1
ST_W23 = ST_W01 + C       # cols  384:640  rows 0:2 = W^T rows 2,3 (/512)
ST_BIAS = ST_W23 + C      # cols  640:896  row 0 = bias
ST_KB = ST_BIAS + C       # cols  896:960  row 0 = k(box)*2^20
ST_ONES = ST_KB + NBOX    # cols  960:1024 row 0 = 1.0
ST_C31 = ST_ONES + NBOX   # col  1024      = [256; 16384] offset coefficients
ST_ONE1 = ST_C31 + 1      # col  1025      = [1; 0]
ST_COLS = ST_ONE1 + 1

USE_MOD_FLOOR = False  # floor via f32 mod-1 ALU op instead of round-magic

_CACHE = {}  # repeat -> compiled program (input-agnostic)


def _emit_floor(nc, pool, out_ap, v_ap, shape, tag):
    """out = floor(v) for v >= 0, bit-exact IEEE f32 (no HW floor op)."""
    r = pool.tile(shape, mybir.dt.float32, tag=f"flr_r{tag}")
    m = pool.tile(shape, mybir.dt.float32, tag=f"flr_m{tag}")
    nc.vector.tensor_scalar(
        out=r[:], in0=v_ap, scalar1=MAGIC, scalar2=MAGIC,
        op0=mybir.AluOpType.add, op1=mybir.AluOpType.subtract,
    )
    nc.vector.tensor_tensor(out=m[:], in0=r[:], in1=v_ap, op=mybir.AluOpType.is_gt)
    nc.vector.tensor_tensor(out=out_ap, in0=r[:], in1=m[:], op=mybir.AluOpType.subtract)


def _build_program(repeat):
    nc = bacc.Bacc("TRN2", num_devices=NCORES, debug=False, enable_asserts=False)

    inp = nc.dram_tensor("inp", [K, FM, FM, C], mybir.dt.float32,
                         kind="ExternalInput")
    st_d = nc.dram_tensor("stage", [2, ST_COLS], mybir.dt.float32,
                          kind="ExternalInput")
    out_d = nc.dram_tensor("out", [NBOX, C], mybir.dt.float32,
                           kind="ExternalOutput")

    f32 = mybir.dt.float32
    i32 = mybir.dt.int32

    # flat element view for the per-partition-offset gather
    view = bass.AP(tensor=inp, offset=0, ap=[[1, NELEM], [1, 1]])

    with tile.TileContext(nc) as tc:
        with tc.tile_pool(name="p", bufs=2) as pool, \
             tc.tile_pool(name="ps", bufs=2, space="PSUM") as psum_pool:
            for _it in range(repeat):
                st = pool.tile([2, ST_COLS], f32)
                nc.sync.dma_start(out=st[:], in_=st_d.ap()[:, :])
                xy = st[0:2, ST_XY:ST_XY + NBOX]
                wh = st[0:2, ST_WH:ST_WH + NBOX]

                # ---- cells: v8 = (xy + floor(wh/2))/8 ; cell = floor(v8)
                shp = [2, NBOX]
                pc = pool.tile(shp, f32)   # pixel center
                v8 = pool.tile(shp, f32)   # pc / 8
                cell = pool.tile(shp, f32)
                fracxy = pool.tile(shp, f32)
                if USE_MOD_FLOOR:
                    vh = pool.tile(shp, f32)
                    nc.vector.tensor_scalar_mul(out=vh[:], in0=wh, scalar1=0.5)
                    fh = pool.tile(shp, f32)
                    nc.vector.tensor_scalar(
                        out=fh[:], in0=wh, scalar1=0.5, scalar2=1.0,
                        op0=mybir.AluOpType.mult, op1=mybir.AluOpType.mod)
                    halfwh = pool.tile(shp, f32)
                    nc.vector.tensor_tensor(out=halfwh[:], in0=vh[:], in1=fh[:],
                                            op=mybir.AluOpType.subtract)
                    nc.vector.tensor_tensor(out=pc[:], in0=xy, in1=halfwh[:],
                                            op=mybir.AluOpType.add)
                    nc.vector.tensor_scalar_mul(out=v8[:], in0=pc[:],
                                                scalar1=0.125)
                    fr8 = pool.tile(shp, f32)
                    nc.vector.tensor_scalar(
                        out=fr8[:], in0=pc[:], scalar1=8.0, scalar2=0.125,
                        op0=mybir.AluOpType.mod, op1=mybir.AluOpType.mult)
                    # fr8 = (pc mod 8)/8 = v8 - floor(v8) = label frac
                    cellr = pool.tile(shp, f32)
                    nc.vector.tensor_tensor(out=cellr[:], in0=v8[:], in1=fr8[:],
                                            op=mybir.AluOpType.subtract)
                    nc.vector.tensor_scalar(
                        out=cell[:], in0=cellr[:], scalar1=0.0,
                        scalar2=float(FM - 1),
                        op0=mybir.AluOpType.max, op1=mybir.AluOpType.min)
                    nc.vector.tensor_tensor(out=fracxy[:], in0=v8[:],
                                            in1=cell[:],
                                            op=mybir.AluOpType.subtract)
                else:
                    vh = pool.tile(shp, f32)
                    nc.vector.tensor_scalar_mul(out=vh[:], in0=wh, scalar1=0.5)
                    halfwh = pool.tile(shp, f32)
                    _emit_floor(nc, pool, halfwh[:], vh[:], shp, "h")
                    nc.vector.tensor_tensor(out=pc[:], in0=xy, in1=halfwh[:],
                                            op=mybir.AluOpType.add)
                    nc.vector.tensor_scalar_mul(out=v8[:], in0=pc[:],
                                                scalar1=0.125)
                    cellr = pool.tile(shp, f32)
                    _emit_floor(nc, pool, cellr[:], v8[:], shp, "c")
                    nc.vector.tensor_scalar(
                        out=cell[:], in0=cellr[:], scalar1=0.0,
                        scalar2=float(FM - 1),
                        op0=mybir.AluOpType.max, op1=mybir.AluOpType.min)
                    nc.vector.tensor_tensor(out=fracxy[:], in0=v8[:],
                                            in1=cell[:],
                                            op=mybir.AluOpType.subtract)

                # ---- gather offsets: 256*cx + 16384*cy + 2^20*k --------
                base_ps = psum_pool.tile([NBOX, 1], f32, space="PSUM")
                nc.tensor.matmul(out=base_ps[:], lhsT=cell[:],
                                 rhs=st[0:2, ST_C31:ST_C31 + 1],
                                 start=True, stop=False)
                nc.tensor.matmul(out=base_ps[:], lhsT=st[0:1, ST_KB:ST_KB + NBOX],
                                 rhs=st[0:1, ST_ONE1:ST_ONE1 + 1],
                                 start=False, stop=True)
                base_i = pool.tile([NBOX, 1], i32)
                nc.vector.tensor_copy(out=base_i[:], in_=base_ps[:])

                # ---- gather: one offset per partition, 1 KiB payload ---
                fv = pool.tile([NBOX, C], f32)
                nc.gpsimd.indirect_dma_start(
                    out=fv[:],
                    out_offset=None,
                    in_=view,
                    in_offset=bass.IndirectOffsetOnAxis(ap=base_i[:, 0:1], axis=0),
                )

                # ---- label linear (w/512,h/512 folded into staged W^T) -
                acc = psum_pool.tile([NBOX, C], f32, space="PSUM")
                nc.tensor.matmul(out=acc[:], lhsT=fracxy[:],
                                 rhs=st[0:2, ST_W01:ST_W01 + C],
                                 start=True, stop=False)
                nc.tensor.matmul(out=acc[:], lhsT=wh,
                                 rhs=st[0:2, ST_W23:ST_W23 + C],
                                 start=False, stop=False)
                nc.tensor.matmul(out=acc[:], lhsT=st[0:1, ST_ONES:ST_ONES + NBOX],
                                 rhs=st[0:1, ST_BIAS:ST_BIAS + C],
                                 start=False, stop=True)

                outt = pool.tile([NBOX, C], f32)
                nc.vector.tensor_tensor(out=outt[:], in0=fv[:], in1=acc[:],
                                        op=mybir.AluOpType.add)
                nc.scalar.dma_start(out=out_d.ap()[:, :], in_=outt[:, :])

    nc.compile()
    return nc


def _get_compiled(repeat=1):
    if repeat not in _CACHE:
        _CACHE[repeat] = _build_program(repeat)
    return _CACHE[repeat]


def _make_stage(bb, W, b):
    """bb: [NBOX, 4] this core's boxes."""
    st = np.zeros((2, ST_COLS), np.float32)
    bbT = bb.T  # [4, 64]
    st[0:2, ST_XY:ST_XY + NBOX] = bbT[0:2]
    st[0:2, ST_WH:ST_WH + NBOX] = bbT[2:4]
    wt = np.asarray(W, np.float32).T  # [4, 256]
    st[0:2, ST_W01:ST_W01 + C] = wt[0:2]
    st[0:2, ST_W23:ST_W23 + C] = wt[2:4] * np.float32(1.0 / 512.0)
    st[0, ST_BIAS:ST_BIAS + C] = np.asarray(b, np.float32)
    st[0, ST_KB:ST_KB + NBOX] = np.repeat(
        np.arange(K, dtype=np.float32) * (HW * C), N)
    st[0, ST_ONES:ST_ONES + NBOX] = 1.0
    st[0, ST_C31] = float(C)        # cx coefficient
    st[1, ST_C31] = float(FM * C)   # cy coefficient
    st[0, ST_ONE1] = 1.0
    return st


def _make_in_maps(input, bboxes, W, b):
    inp = np.asarray(input, np.float32)
    bbx = np.asarray(bboxes, np.float32)
    in_maps = []
    for core in range(NCORES):
        shard = inp[core * K:(core + 1) * K]  # [K, C, FM, FM]
        shard = np.ascontiguousarray(shard.transpose(0, 2, 3, 1))  # NHWC
        in_maps.append({
            "inp": shard,
            "stage": _make_stage(bbx[core].reshape(NBOX, 4), W, b),
        })
    return in_maps


def run(input, bboxes, W, b, trace=False, repeat=1):
    """Returns (full_output [B,K,N,C] f32, BassKernelResults)."""
    nc = _get_compiled(repeat)
    res = bass_utils.run_bass_kernel_spmd(
        nc, _make_in_maps(input, bboxes, W, b),
        core_ids=list(range(NCORES)), trace=trace,
    )
    out = np.stack([r["out"] for r in res.results], axis=0)  # [8, 64, 256]
    return out.reshape(B, K, N, C), res


def kernel(input, bboxes, W, b):
    out, _ = run(input, bboxes, W, b, trace=False)
    return out
